# revision 39
# baseline (speedup 1.0000x reference)
"""Trainium2 Bass kernel for fused BERT-CRF-NER word_embedding + sigmoid.

Math (per batch row):
  inner[t]   = 1 <= t <= L-2          (L = valid length from contiguous mask)
  starts     = first_label_mask & inner
  word_id[t] = cumsum(starts) - 1     (-1 outside inner)
  wv[k]      = mean of token_features[t] over word_id[t] == k
  emission   = sigmoid(wv @ W.T + b)  (empty word slots -> sigmoid(b))

Ragged-aware restructuring: valid tokens are a contiguous prefix (only
positions 0..L-2 can matter; wid masks out t=0 and t>L-2), and only ~50% of
all tokens are valid, so instead of streaming the full [S, D] block per row:
  1) host-side, rows are assigned to cores/slots balancing total valid
     tokens (the sharding decision: 8 rows per core, sorted into 8 length-
     octile "slots" so every core's slot j has the same static token
     capacity num_j and word capacity K_j),
  2) each slot loads ONLY its first num_j tokens - a contiguous prefix is a
     static rectangular access pattern, so a plain SWDGE cast-DMA (fp32 ->
     bf16 in flight) moves exactly the needed bytes in the interleaved
     layout token t = 256c + 2p + u (6KB per descriptor). Token slots
     beyond a row's valid region hold real in-row floats whose word id is
     -1, so the membership matrix zeroes them; SBUF partitions beyond a
     slot's capacity are never written OR read (every consumer slices
     [0:p1]),
  3) the membership matrix M[t, k] = (word_id[t] == k) turns the
     segment-sum into PE matmuls Z^T = X^T M with per-slot superblock
     count B2_j and word capacity K_j,
  4) logits^T = W^T Z^T + b x counts (bias pre-scaled by the clamped
     counts so the final sigmoid fuses the 1/count into its scale operand:
     sigmoid((Z^T W + b c) / c) = sigmoid(Z^T W / c + b)).
Sharding: data parallel, 8 batch rows per core across 8 cores, with the
row->core assignment chosen to balance per-core valid-token counts.
"""

from contextlib import ExitStack

import numpy as np

import concourse.bass as bass
import concourse.tile as tile
from concourse import bacc, mybir
from concourse.bass_utils import run_bass_kernel_spmd

B, S, D, NL = 64, 512, 768, 10
N_CORES = 8
RPC = B // N_CORES  # batch rows (slots) per core
DC = D // 128       # feature chunks of 128

f32 = mybir.dt.float32
bf16 = mybir.dt.bfloat16
i32 = mybir.dt.int32
Alu = mybir.AluOpType
Act = mybir.ActivationFunctionType


def _plan(input_mask, first_label_mask):
    """Host-side sharding plan from the masks.

    Returns (perm[core][slot] -> original row, num_js, K_js).
    num_j = token capacity of slot j (multiple of 16, covers tokens
    0..L-2 of every row assigned to the slot), K_j = word-slot capacity.
    Slots are length octiles, so capacities are tight; cores are balanced
    on total tokens.
    """
    im = np.asarray(input_mask)
    fm = np.asarray(first_label_mask)
    L = im.sum(axis=1)
    n = np.clip(L - 1, 2, S)  # tokens 0..L-2 kept per row
    pos = np.arange(S)[None, :]
    inner = (im > 0) & (pos >= 1) & (pos <= (L - 2)[:, None])
    words = ((fm > 0) & inner).sum(axis=1)

    order = np.argsort(-n, kind="stable")
    perm = np.zeros((N_CORES, RPC), dtype=np.int64)
    core_tot = np.zeros(N_CORES, dtype=np.int64)
    num_js, K_js = [], []
    for j in range(RPC):
        rows = order[j * N_CORES : (j + 1) * N_CORES]
        # largest row in this octile -> currently lightest core
        rows = rows[np.argsort(-n[rows], kind="stable")]
        cores = np.argsort(core_tot, kind="stable")
        for r, c in zip(rows, cores):
            perm[c][j] = r
            core_tot[c] += n[r]
        maxn = max(int(n[rows].max()), 16)
        num_js.append(128 * ((maxn + 127) // 128))
        maxw = max(int(words[rows].max()), 1)
        K_js.append(16 * ((maxw + 15) // 16))
    return perm, tuple(num_js), tuple(K_js)


def _build_nc(num_js, K_js):
    C_js = [num // 128 for num in num_js]
    KC_js = [(K + 127) // 128 for K in K_js]
    KMAX = max(128, max(K_js))

    nc = bacc.Bacc("TRN2", target_bir_lowering=False, debug=False)
    x_d = nc.dram_tensor("x", [RPC, S, D], f32, kind="ExternalInput")
    mk_d = nc.dram_tensor("mk", [RPC, 3, S], i32, kind="ExternalInput")
    wtb_d = nc.dram_tensor(
        "wtb", [128, DC * NL + NL + 1], f32, kind="ExternalInput"
    )
    iokp_d = nc.dram_tensor("iokp", [128, KMAX], bf16, kind="ExternalInput")
    out_d = nc.dram_tensor("out", [RPC, S, NL], f32, kind="ExternalOutput")

    with tile.TileContext(nc) as tc, ExitStack() as ctx:
        const = ctx.enter_context(tc.tile_pool(name="const", bufs=1))
        xp = ctx.enter_context(tc.tile_pool(name="xp", bufs=1))
        zsp = ctx.enter_context(tc.tile_pool(name="zsp", bufs=3))
        rsp = ctx.enter_context(tc.tile_pool(name="rsp", bufs=2))
        obp = ctx.enter_context(tc.tile_pool(name="obp", bufs=2))
        ztp = ctx.enter_context(
            tc.tile_pool(name="ztp", bufs=2, space=bass.MemorySpace.PSUM)
        )
        lgp = ctx.enter_context(
            tc.tile_pool(name="lgp", bufs=1, space=bass.MemorySpace.PSUM)
        )
        ctp = ctx.enter_context(
            tc.tile_pool(name="ctp", bufs=1, space=bass.MemorySpace.PSUM)
        )
        tpp = ctx.enter_context(
            tc.tile_pool(name="tpp", bufs=2, space=bass.MemorySpace.PSUM)
        )

        def chunks(j):
            return list(range(C_js[j]))

        # ---- small loads FIRST, on the same SWDGE queue as the X loads:
        # a separate queue gets starved by the X stream (packet round-robin
        # is heavily skewed), which stalled compute start until stream end.
        # In-queue FIFO order guarantees these land in the first ~1us.
        mk_t = const.tile([RPC, 3, S], i32)
        nc.gpsimd.dma_start(mk_t[:], mk_d[:, :, :])
        iokp = const.tile([128, KMAX], bf16)
        nc.gpsimd.dma_start(iokp[:], iokp_d[:, :])
        wtb = const.tile([128, DC * NL + NL + 1], f32)
        nc.gpsimd.dma_start(wtb[:], wtb_d[:, :])

        iok = iokp
        wt_f = wtb[:, 0 : DC * NL].rearrange("p (j l) -> p j l", l=NL)
        b_sb = wtb[0:1, DC * NL : DC * NL + NL]

        # ---- X loads: ONE plain SWDGE cast-DMA per slot, full 128
        # partitions (partial-partition DMAs use only a subset of the SDMA
        # engines and run well below line rate). Token t = 128c + p; only
        # the first num_j tokens of each slot move; fp32 -> bf16 in flight.
        xbf_ts = {}
        for j in range(RPC):
            C, num = C_js[j], num_js[j]
            xbf_ts[j] = xp.tile([128, C, D], bf16, name=f"xbf_{j}")
            nc.gpsimd.dma_start(
                xbf_ts[j][:],
                x_d[j, 0:num, :].rearrange("(c p) d -> p c d", p=128),
            )

        # ---- mask pipeline: word ids per token, all RPC rows at once ----
        imf = const.tile([RPC, S], f32)
        nc.vector.tensor_copy(imf[:], mk_t[:, 0, :])
        fmf = const.tile([RPC, S], f32)
        nc.vector.tensor_copy(fmf[:], mk_t[:, 1, :])
        posf = const.tile([RPC, S], f32)
        nc.vector.tensor_copy(posf[:], mk_t[:, 2, :])

        L8 = const.tile([RPC, 1], f32)
        nc.vector.tensor_reduce(L8[:], imf[:], axis=mybir.AxisListType.X, op=Alu.add)
        lm2 = const.tile([RPC, 1], f32)
        nc.vector.tensor_scalar_add(lm2[:], L8[:], -2.0)

        inner = const.tile([RPC, S], f32)
        nc.vector.tensor_scalar(
            inner[:], posf[:], lm2[:, 0:1], None, op0=Alu.is_le
        )
        nc.vector.memset(inner[:, 0:1], 0.0)  # position 0 ([CLS]) excluded

        starts = const.tile([RPC, S], f32)
        nc.vector.tensor_mul(starts[:], fmf[:], inner[:])
        widr = const.tile([RPC, S], f32)
        nc.vector.tensor_tensor_scan(
            widr[:], starts[:], starts[:], 0.0, op0=Alu.add, op1=Alu.bypass
        )
        wid = const.tile([RPC, S], f32)
        nc.vector.tensor_mul(wid[:], widr[:], inner[:])
        nc.vector.tensor_scalar_add(wid[:], wid[:], -1.0)

        # DVE consts
        ident = const.tile([128, 128], f32)
        nc.vector.tensor_scalar(
            ident[:], iok[:, 0:128], wtb[:, DC * NL + NL :], None,
            op0=Alu.is_equal,
        )
        ones_r = const.tile([128, 1], bf16)
        nc.vector.memset(ones_r[:], 1.0)
        ones1 = const.tile([1, 128], f32)
        nc.vector.memset(ones1[:], 1.0)
        wt = const.tile([128, DC, NL], bf16)
        nc.vector.tensor_copy(wt[:], wt_f)

        # sigmoid(b) broadcast [128, 3, NL] for the constant slot region
        sigb_row = const.tile([1, NL], f32)
        nc.scalar.activation(sigb_row[:], b_sb, Act.Sigmoid)
        sigb_ps = tpp.tile([128, 16], f32, tag="tp")
        nc.tensor.matmul(sigb_ps[:, 0:NL], ones1[0:1, :], sigb_row[0:1, :])
        sigb2 = const.tile([128, 3, NL], f32)
        nc.scalar.copy(sigb2[:, 0, :], sigb_ps[:, 0:NL])
        nc.scalar.copy(sigb2[:, 1, :], sigb_ps[:, 0:NL])
        nc.scalar.copy(sigb2[:, 2, :], sigb_ps[:, 0:NL])

        # word ids on token partitions in the shared t = 256c + 2p + u
        # layout: widT[:, c, u, r] = wid[r, 256c + 2p + u]
        widT = const.tile([128, S // 128, RPC], f32)
        wid_v = wid[:].rearrange("r (c p) -> r c p", c=S // 128)
        for c in range(S // 128):
            tp_ps = tpp.tile([128, 16], f32, tag="tp")
            nc.tensor.transpose(
                tp_ps[:, 0:RPC], wid_v[:, c, :], ident[0:RPC, 0:RPC]
            )
            nc.vector.tensor_copy(widT[:, c, :], tp_ps[:, 0:RPC])

        countsT = const.tile([128, RPC, 2], f32)
        nc.vector.memset(countsT[:], 0.0)
        recipT = const.tile([128, RPC, 2], f32)

        # membership matrices for ALL slots up front: they depend only on
        # the masks, so the DVE builds them while the X data is in flight
        # and the PE never waits on a just-in-time m build
        m_ts = {}
        for j in range(RPC):
            K = K_js[j]
            m_t = const.tile([128, S // 128, KMAX], bf16, name=f"m_{j}")
            for c in chunks(j):
                nc.vector.tensor_scalar(
                    m_t[:, c, 0:K], iok[:, 0:K], widT[:, c, j : j + 1],
                    None, op0=Alu.is_equal,
                )
            m_ts[j] = m_t

        # ---- heavy per-slot pipeline, software-pipelined ----------------
        zs_ts = {}
        ct_sbs = {}

        def counts(j):
            K, KC = K_js[j], KC_js[j]
            ch = chunks(j)
            m_t = m_ts[j]
            ct_ps = ctp.tile([1, KMAX], f32, tag="ct")
            for i, c in enumerate(ch):
                nc.tensor.matmul(
                    ct_ps[:, 0:K], ones_r[:], m_t[:, c, 0:K],
                    start=(i == 0), stop=(i == len(ch) - 1),
                )
            ct_sb = const.tile([1, KMAX], f32, name=f"ct_{j}")
            nc.vector.tensor_scalar_max(ct_sb[:, 0:K], ct_ps[:, 0:K], 1.0)
            ct_sbs[j] = ct_sb
            for c2 in range(KC):
                w = min(128, K - c2 * 128)
                tp_ps = tpp.tile([128, 16], f32, tag="tp")
                nc.tensor.transpose(
                    tp_ps[0:w, 0:1], ct_sb[0:1, c2 * 128 : c2 * 128 + w],
                    ident[0:1, 0:1],
                )
                nc.scalar.copy(countsT[0:w, j, c2 : c2 + 1], tp_ps[0:w, 0:1])
            nc.vector.reciprocal(recipT[:, j, :], countsT[:, j, :])

        def stage1(j):
            K = K_js[j]
            ch = chunks(j)
            m_t = m_ts[j]
            zs_t = zsp.tile([128, DC, KMAX], bf16, tag="zs")
            for h in range(2):
                # per-jj stride padded to 256 so each slice stays in one bank
                zt_ps = ztp.tile([128, DC // 2, 256], f32, tag="zt")
                for jj in range(DC // 2):
                    j6 = h * (DC // 2) + jj
                    for i, c in enumerate(ch):
                        nc.tensor.matmul(
                            zt_ps[:, jj, 0:K],
                            xbf_ts[j][:, c, j6 * 128 : (j6 + 1) * 128],
                            m_t[:, c, 0:K],
                            start=(i == 0),
                            stop=(i == len(ch) - 1),
                        )
                if h == 0:
                    nc.scalar.copy(
                        zs_t[:, 0 : DC // 2, 0:K], zt_ps[:, :, 0:K]
                    )
                else:
                    nc.vector.tensor_copy(
                        zs_t[:, DC // 2 : DC, 0:K], zt_ps[:, :, 0:K]
                    )
            zs_ts[j] = zs_t

        def stage2_tail(j):
            K, KC = K_js[j], KC_js[j]
            ch = chunks(j)
            zs_t = zs_ts.pop(j)
            ct_sb = ct_sbs[j]

            # logits^T[l, k] = sum_d W^T[d, l] Z^T[d, k] + b[l] * counts[k]
            # (bias pre-scaled by clamped counts: the sigmoid below divides
            # everything by counts via its scale operand)
            lg_ps = lgp.tile([NL, KMAX], f32, tag="lg")
            for j6 in range(DC):
                nc.tensor.matmul(
                    lg_ps[:, 0:K], wt[:, j6, :], zs_t[:, j6, 0:K],
                    start=(j6 == 0), stop=False,
                )
            nc.tensor.matmul(
                lg_ps[:, 0:K], b_sb, ct_sb[0:1, 0:K],
                start=False, stop=True,
            )

            lg_sb = rsp.tile([NL, KMAX], f32, tag="logit")
            nc.scalar.copy(lg_sb[:, 0:K], lg_ps[:, 0:K])

            # tail: transpose logits, fused sigmoid(x / count), one store
            row_out = obp.tile([128, S // 128, NL], f32, tag="row")
            if KC == 2:
                w1 = K - 128
                if w1 < 128:
                    nc.scalar.copy(row_out[:, 1, :], sigb2[:, 0, :])
                nc.scalar.copy(row_out[:, 2:4, :], sigb2[:, 0:2, :])
            else:
                if K < 128:
                    nc.scalar.copy(row_out[:, 0, :], sigb2[:, 0, :])
                nc.scalar.copy(row_out[:, 1:4, :], sigb2[:])
            for c2 in range(KC):
                w = min(128, K - c2 * 128)
                tp_ps = tpp.tile([128, 16], f32, tag="tp")
                nc.tensor.transpose(
                    tp_ps[0:w, 0:NL],
                    lg_sb[:, c2 * 128 : c2 * 128 + w],
                    ident[0:NL, 0:NL],
                )
                nc.scalar.activation(
                    row_out[0:w, c2, :], tp_ps[0:w, 0:NL], Act.Sigmoid,
                    scale=recipT[0:w, j, c2 : c2 + 1],
                )
            nc.sync.dma_start(
                out_d[j].rearrange("(c p) l -> p c l", p=128), row_out[:]
            )

        for j in range(RPC):
            counts(j)
            if j > 0:
                stage2_tail(j - 1)
            stage1(j)
        stage2_tail(RPC - 1)

    nc.compile()
    return nc


_NC_CACHE: dict = {}


def make_in_maps(ins):
    import ml_dtypes  # numpy has no native bf16

    x = np.ascontiguousarray(ins["token_features"], dtype=np.float32)
    im = np.ascontiguousarray(ins["input_mask"], dtype=np.int32)
    fm = np.ascontiguousarray(ins["first_label_mask"], dtype=np.int32)
    W = np.asarray(ins["W"], dtype=np.float32)
    bb = np.asarray(ins["b"], dtype=np.float32).reshape(NL)

    perm, num_js, K_js = _plan(im, fm)
    KMAX = max(128, max(K_js))
    iokp = np.ascontiguousarray(
        np.tile(np.arange(KMAX, dtype=np.float32), (128, 1)).astype(
            ml_dtypes.bfloat16
        )
    )
    # wt packed [128, 60+10+1]: wtb[p, j*NL+l] = W.T[j*128+p, l];
    # b on row 0; last col = partition iota (f32 scalar for is_equal)
    wtb = np.zeros((128, DC * NL + NL + 1), dtype=np.float32)
    wtb[:, 0 : DC * NL] = (
        W.T.reshape(DC, 128, NL).transpose(1, 0, 2).reshape(128, DC * NL)
    )
    wtb[0, DC * NL : DC * NL + NL] = bb
    wtb[:, DC * NL + NL] = np.arange(128, dtype=np.float32)
    wtb = np.ascontiguousarray(wtb)
    posi = np.tile(np.arange(S, dtype=np.int32), (RPC, 1))

    in_maps = []
    for i in range(N_CORES):
        rows = perm[i]
        mk = np.stack([im[rows], fm[rows], posi], axis=1)  # [RPC, 3, S]
        in_maps.append(
            {
                "x": np.ascontiguousarray(x[rows]),
                "mk": np.ascontiguousarray(mk),
                "wtb": wtb,
                "iokp": iokp,
            }
        )
    return in_maps, perm, num_js, K_js


def assemble_out(res, perm):
    out = np.zeros((B, S, NL), dtype=np.float32)
    for i in range(N_CORES):
        out[perm[i]] = res.results[i]["out"]
    return out


def kernel(token_features, input_mask, first_label_mask, W, b):
    ins = {
        "token_features": token_features,
        "input_mask": input_mask,
        "first_label_mask": first_label_mask,
        "W": W,
        "b": b,
    }
    in_maps, perm, num_js, K_js = make_in_maps(ins)
    key = (num_js, K_js)
    if key not in _NC_CACHE:
        _NC_CACHE[key] = _build_nc(num_js, K_js)
    nc = _NC_CACHE[key]
    res = run_bass_kernel_spmd(nc, in_maps, list(range(N_CORES)))
    return assemble_out(res, perm)


if __name__ == "__main__":
    rng = np.random.default_rng(0)
    tf = rng.standard_normal((B, S, D), dtype=np.float32)
    lengths = rng.integers(16, S + 1, size=(B,))
    pos = np.arange(S)[None, :]
    im = (pos < lengths[:, None]).astype(np.int32)
    fm = ((rng.random((B, S)) < 0.4) & (im > 0)).astype(np.int32)
    fm[:, 1] = 1
    W = (rng.standard_normal((NL, D)) * 0.02).astype(np.float32)
    b = np.zeros(NL, np.float32)
    out = kernel(
        token_features=tf, input_mask=im, first_label_mask=fm, W=W, b=b
    )
    print(out.shape, out.dtype)
